# revision 12
# baseline (speedup 1.0000x reference)
"""Trainium2 Bass kernel for nn_BertEmbedding 'bissect' pooling head.

Reference computation (per batch b, token t):
    hs   = hidden_states[1:13]                 # drop embedding layer -> 12 layers
    v    = mean_n hs[n,b,t,:]                  # (768,)
    s_n  = v . hs[n,b,t,:]                     # 12 scores
    p    = softmax(s)                          # over the 12 layers
    final= sum_n p_n * hs[n,b,t,:]             # (768,)
    pooled[b,:] = max_t final[b,t,:]*mask[b,t]
    logits = pooled @ W.T + b                  # (b, 2)

Sharding: pure data parallelism over batch (32 batches -> 4 per core x 8 cores).

Per-core dataflow (memory-bound; ~75.5 MB HBM reads per core):
  - hidden_states loaded once, cast f32->fp16 in the DMA (SWDGE casting DMA).
  - u = sum_n hs_n on the PE via identity-matmul PSUM accumulation (fp16 in,
    f32 accumulate).
  - scores_n = sum_h (hs_n/12)*u fused on DVE via scalar_tensor_tensor with
    accum_out (one instruction per layer).
  - softmax on ACT (exp with per-partition bias + fused denominator accum).
  - final = sum_n p_n*hs_n on the PE via diag(p_n)-matmul PSUM accumulation.
  - masked token-max with one fused DVE op per subtile; cross-partition max
    via PE transpose + DVE free-dim reduce; tiny final linear via ones-matmul.
"""

import os
import sys
from contextlib import ExitStack

import numpy as np

for _p in ("/opt/trn_rl_repo",):
    if _p not in sys.path:
        sys.path.insert(0, _p)

import concourse.bacc as bacc
import concourse.bass as bass
import concourse.mybir as mybir
import concourse.tile as tile
from concourse.bass_utils import run_bass_kernel_spmd

F32 = mybir.dt.float32
F16 = mybir.dt.float16
AX = mybir.AxisListType
OP = mybir.AluOpType
ACT = mybir.ActivationFunctionType

NCORES = 8
L = 12          # layers used (hidden_states[1:13])
BFULL, TSEQ, H = 32, 512, 768
B = BFULL // NCORES            # batches per core = 4
NL = 2                         # num labels
HC = H // 128                  # h chunks of 128 = 6
X = 2                          # 128-token subtiles per DMA unit
TOK = 128 * X                  # tokens per unit = 256
NUNIT = TSEQ // TOK            # units per batch = 2
NEG_INF = -3.0e38


def _build_nc(reps=1):
    nc = bacc.Bacc("TRN2", target_bir_lowering=False, debug=False,
                   num_devices=NCORES)
    hs_d = nc.declare_dram_parameter("hs", [L, B, TSEQ, H], F32, isOutput=False)
    mask_d = nc.declare_dram_parameter("mask", [B, TSEQ], F32, isOutput=False)
    wres_d = nc.declare_dram_parameter("wres", [128, NL * HC], F32, isOutput=False)
    bres_d = nc.declare_dram_parameter("bres", [1, B * NL], F32, isOutput=False)
    id16_d = nc.declare_dram_parameter("id16", [128, 128], F16, isOutput=False)
    id32_d = nc.declare_dram_parameter("id32", [128, 128], F32, isOutput=False)
    out_d = nc.declare_dram_parameter("out", [1, B * NL], F32, isOutput=True)

    with tile.TileContext(nc) as tc:
        with ExitStack() as ctx:
            _body(ctx, tc, nc, hs_d, mask_d, wres_d, bres_d, id16_d, id32_d,
                  out_d, reps)
    nc.compile()
    return nc


def _body(ctx, tc, nc, hs_d, mask_d, wres_d, bres_d, id16_d, id32_d, out_d,
          reps=1):
    singles = ctx.enter_context(tc.tile_pool(name="singles", bufs=1))
    hs_pool = ctx.enter_context(tc.tile_pool(name="hs", bufs=2))
    work = ctx.enter_context(tc.tile_pool(name="work", bufs=2))
    small = ctx.enter_context(tc.tile_pool(name="small", bufs=4))
    diags = ctx.enter_context(tc.tile_pool(name="diags", bufs=4))
    batchp = ctx.enter_context(tc.tile_pool(name="batchp", bufs=2))
    psum_u = ctx.enter_context(tc.tile_pool(name="psum_u", bufs=1, space="PSUM"))
    psum_f = ctx.enter_context(tc.tile_pool(name="psum_f", bufs=2, space="PSUM"))
    psum_tr = ctx.enter_context(tc.tile_pool(name="psum_tr", bufs=2, space="PSUM"))

    id16 = singles.tile([128, 128], F16)
    nc.sync.dma_start(out=id16, in_=id16_d[:, :])
    id32 = singles.tile([128, 128], F32)
    nc.sync.dma_start(out=id32, in_=id32_d[:, :])
    wres = singles.tile([128, NL * HC], F32)
    nc.sync.dma_start(out=wres, in_=wres_d[:, :])
    bres = singles.tile([1, B * NL], F32)
    nc.sync.dma_start(out=bres, in_=bres_d[:, :])
    ones = singles.tile([128, 1], F32)
    nc.vector.memset(ones, 1.0)
    partials = singles.tile([128, B * NL], F32)
    logits_sb = singles.tile([1, B * NL], F32)

    chunks = [(0, 512), (512, 256)]

    def batch_sweep():
        for bb in range(B):
            _batch(tc, nc, hs_d, mask_d, hs_pool, work, small, diags, batchp,
                   psum_u, psum_f, psum_tr, id16, id32, wres, partials, chunks,
                   bb)

    if reps == 1:
        batch_sweep()
    else:
        with tc.For_i(0, reps, 1):
            batch_sweep()

    # ---- reduce partials over partitions with a ones-matmul; add bias ----
    lg_ps = psum_tr.tile([1, B * NL], F32, tag="tr")
    nc.tensor.matmul(lg_ps, ones, partials, start=True, stop=True)
    nc.vector.tensor_add(logits_sb, lg_ps, bres)
    nc.sync.dma_start(out=out_d[:, :], in_=logits_sb)


def _batch(tc, nc, hs_d, mask_d, hs_pool, work, small, diags, batchp, psum_u,
           psum_f, psum_tr, id16, id32, wres, partials, chunks, bb):
    if True:
        maxacc = batchp.tile([128, H], F32, tag="maxacc")
        nc.gpsimd.memset(maxacc, NEG_INF)

        for hh in range(NUNIT):
            # ---- load 12 layer tiles for 256 tokens, casting f32->fp16 ----
            hs16 = []
            for n in range(L):
                t = hs_pool.tile([128, X, H], F16, tag=f"hs{n}")
                src = hs_d[n, bb, hh * TOK:(hh + 1) * TOK, :].rearrange(
                    "(x p) h -> p x h", p=128)
                nc.gpsimd.dma_start(out=t, in_=src)
                hs16.append(t)
            msk = small.tile([128, X], F32, tag="msk")
            nc.sync.dma_start(
                out=msk,
                in_=mask_d[bb, hh * TOK:(hh + 1) * TOK].rearrange("(x p) -> p x", p=128))

            for x in range(X):
                # ---- u = sum_n hs_n  (PE identity accumulation) ----
                u_ps = psum_u.tile([128, H], F32, tag="u")
                for c0, cw in chunks:
                    for n in range(L):
                        nc.tensor.matmul(
                            u_ps[:, c0:c0 + cw], id16, hs16[n][:, x, c0:c0 + cw],
                            start=(n == 0), stop=(n == L - 1))
                u16 = work.tile([128, H], F16, tag="u16")
                nc.scalar.copy(u16, u_ps)

                # ---- scores_n = sum_h (hs_n/12)*u  (fused STT + accum) ----
                scores = small.tile([128, L], F32, tag="scores")
                scratch = work.tile([128, H], F16, tag="scratch")
                for n in range(L):
                    nc.vector.scalar_tensor_tensor(
                        out=scratch, in0=hs16[n][:, x, :], scalar=1.0 / L,
                        in1=u16, op0=OP.mult, op1=OP.mult,
                        accum_out=scores[:, n:n + 1])

                # ---- softmax over the 12 layers ----
                mx = small.tile([128, 1], F32, tag="mx")
                nc.vector.tensor_reduce(out=mx, in_=scores, axis=AX.X, op=OP.max)
                negmx = small.tile([128, 1], F32, tag="negmx")
                nc.vector.tensor_scalar_mul(negmx, mx, -1.0)
                exps = small.tile([128, L], F32, tag="exps")
                denom = small.tile([128, 1], F32, tag="denom")
                nc.scalar.activation(out=exps, in_=scores, func=ACT.Exp,
                                     bias=negmx[:, 0:1], scale=1.0,
                                     accum_out=denom[:, 0:1])
                recip = small.tile([128, 1], F32, tag="recip")
                nc.vector.reciprocal(recip, denom)
                pscale = small.tile([128, L], F32, tag="pscale")
                nc.vector.tensor_scalar_mul(pscale, exps, recip[:, 0:1])

                # ---- final = sum_n p_n*hs_n  (PE diag accumulation) ----
                fin_ps = psum_f.tile([128, H], F32, tag="fin")
                for n in range(L):
                    dg = diags.tile([128, 128], F16, tag="diag")
                    nc.scalar.mul(dg, id16, pscale[:, n:n + 1])
                    for c0, cw in chunks:
                        nc.tensor.matmul(
                            fin_ps[:, c0:c0 + cw], dg, hs16[n][:, x, c0:c0 + cw],
                            start=(n == 0), stop=(n == L - 1))

                # ---- masked running max over tokens ----
                nc.vector.scalar_tensor_tensor(
                    out=maxacc, in0=fin_ps, scalar=msk[:, x:x + 1], in1=maxacc,
                    op0=OP.mult, op1=OP.max)

        # ---- pooled[b] = cross-partition max via PE transpose ----
        pooled = batchp.tile([128, HC], F32, tag="pooled")
        for c in range(HC):
            ptr = psum_tr.tile([128, 128], F32, tag="tr")
            nc.tensor.transpose(ptr, maxacc[:, c * 128:(c + 1) * 128], id32)
            nc.vector.tensor_reduce(out=pooled[:, c:c + 1], in_=ptr,
                                    axis=AX.X, op=OP.max)

        # ---- logits partials: sum_h pooled*W per label ----
        sc6 = small.tile([128, HC], F32, tag="sc6")
        for l in range(NL):
            nc.vector.scalar_tensor_tensor(
                out=sc6, in0=pooled, scalar=1.0,
                in1=wres[:, l * HC:(l + 1) * HC], op0=OP.mult, op1=OP.mult,
                accum_out=partials[:, bb * NL + l:bb * NL + l + 1])


_NC_CACHE = None


def _get_nc():
    global _NC_CACHE
    if _NC_CACHE is None:
        _NC_CACHE = _build_nc()
    return _NC_CACHE


def kernel(hidden_states, mask, W, b):
    hidden_states = np.asarray(hidden_states, dtype=np.float32)
    mask = np.asarray(mask, dtype=np.float32)
    W = np.asarray(W, dtype=np.float32)
    b = np.asarray(b, dtype=np.float32)

    nc = _get_nc()

    # wres[p, l*HC+c] = W[l, c*128+p]
    wres = np.ascontiguousarray(
        W.reshape(NL, HC, 128).transpose(2, 0, 1).reshape(128, NL * HC))
    bres = np.ascontiguousarray(np.tile(b, B)[None, :])
    id16 = np.eye(128, dtype=np.float16)
    id32 = np.eye(128, dtype=np.float32)

    in_maps = []
    for ci in range(NCORES):
        in_maps.append({
            "hs": np.ascontiguousarray(hidden_states[1:, ci * B:(ci + 1) * B]),
            "mask": np.ascontiguousarray(mask[ci * B:(ci + 1) * B]),
            "wres": wres,
            "bres": bres,
            "id16": id16,
            "id32": id32,
        })

    trace = bool(int(os.environ.get("BASS_KERNEL_TRACE", "0")))
    res = run_bass_kernel_spmd(nc, in_maps, list(range(NCORES)), trace=trace)
    if trace and res.exec_time_ns is not None:
        print(f"HW exec time: {res.exec_time_ns} ns")
        kernel.last_exec_time_ns = res.exec_time_ns
    out = np.concatenate(
        [res.results[i]["out"].reshape(B, NL) for i in range(NCORES)], axis=0)
    return out


if __name__ == "__main__":
    rng = np.random.default_rng(0)
    hs = rng.standard_normal((13, BFULL, TSEQ, H), dtype=np.float32)
    mask = np.ones((BFULL, TSEQ), dtype=np.float32)
    W = rng.standard_normal((NL, H), dtype=np.float32) * 0.02
    b = np.zeros((NL,), dtype=np.float32)
    out = kernel(hidden_states=hs, mask=mask, W=W, b=b)
    print(out)


# revision 15
# speedup vs baseline: 1.6015x; 1.6015x over previous
"""Trainium2 Bass kernel for nn_BertEmbedding 'bissect' pooling head.

Reference computation (per batch b, token t):
    hs   = hidden_states[1:13]                 # drop embedding layer -> 12 layers
    v    = mean_n hs[n,b,t,:]                  # (768,)
    s_n  = v . hs[n,b,t,:]                     # 12 scores
    p    = softmax(s)                          # over the 12 layers
    final= sum_n p_n * hs[n,b,t,:]             # (768,)
    pooled[b,:] = max_t final[b,t,:]*mask[b,t]
    logits = pooled @ W.T + b                  # (b, 2)

Sharding: pure data parallelism over batch (32 batches -> 4 per core x 8 cores).

Per-core dataflow (memory-bound; ~75.5 MB HBM reads per core):
  - hidden_states loaded once, cast f32->fp16 in the DMA (SWDGE casting DMA).
  - u = sum_n hs_n on the PE via identity-matmul PSUM accumulation (fp16 in,
    f32 accumulate).
  - scores_n = sum_h (hs_n/12)*u fused on DVE via scalar_tensor_tensor with
    accum_out (one instruction per layer).
  - softmax on ACT (exp with per-partition bias + fused denominator accum).
  - final = sum_n p_n*hs_n on the PE via diag(p_n)-matmul PSUM accumulation.
  - masked token-max with one fused DVE op per subtile; cross-partition max
    via PE transpose + DVE free-dim reduce; tiny final linear via ones-matmul.
"""

import os
import sys
from contextlib import ExitStack

import numpy as np

for _p in ("/opt/trn_rl_repo",):
    if _p not in sys.path:
        sys.path.insert(0, _p)

import concourse.bacc as bacc
import concourse.bass as bass
import concourse.mybir as mybir
import concourse.tile as tile
from concourse.bass_utils import run_bass_kernel_spmd

F32 = mybir.dt.float32
F16 = mybir.dt.float16
AX = mybir.AxisListType
OP = mybir.AluOpType
ACT = mybir.ActivationFunctionType

NCORES = 8
L = 12          # layers used (hidden_states[1:13])
BFULL, TSEQ, H = 32, 512, 768
B = BFULL // NCORES            # batches per core = 4
NL = 2                         # num labels
HC = H // 128                  # h chunks of 128 = 6
X = 2                          # 128-token subtiles per DMA unit
TOK = 128 * X                  # tokens per unit = 256
NUNIT = TSEQ // TOK            # units per batch = 2
NEG_INF = -3.0e38
# True: gpsimd casting DMA loads; False: sync f32 loads + ACT cast
CAST_DMA = os.environ.get("K_CAST_DMA", "1") == "1"


def _build_nc(reps=1):
    nc = bacc.Bacc("TRN2", target_bir_lowering=False, debug=False,
                   num_devices=NCORES)
    hs_d = nc.declare_dram_parameter("hs", [L, B, TSEQ, H], F32, isOutput=False)
    mask_d = nc.declare_dram_parameter("mask", [B, TSEQ], F32, isOutput=False)
    wres_d = nc.declare_dram_parameter("wres", [128, NL * HC], F32, isOutput=False)
    bres_d = nc.declare_dram_parameter("bres", [1, B * NL], F32, isOutput=False)
    id16_d = nc.declare_dram_parameter("id16", [128, 128], F16, isOutput=False)
    id32_d = nc.declare_dram_parameter("id32", [128, 128], F32, isOutput=False)
    out_d = nc.declare_dram_parameter("out", [1, B * NL], F32, isOutput=True)

    with tile.TileContext(nc) as tc:
        with ExitStack() as ctx:
            _body(ctx, tc, nc, hs_d, mask_d, wres_d, bres_d, id16_d, id32_d,
                  out_d, reps)
    nc.compile()
    return nc


def _body(ctx, tc, nc, hs_d, mask_d, wres_d, bres_d, id16_d, id32_d, out_d,
          reps=1):
    singles = ctx.enter_context(tc.tile_pool(name="singles", bufs=1))
    hs_pool = ctx.enter_context(tc.tile_pool(name="hs", bufs=2))
    work = ctx.enter_context(tc.tile_pool(name="work", bufs=2))
    small = ctx.enter_context(tc.tile_pool(name="small", bufs=4))
    diags = ctx.enter_context(tc.tile_pool(name="diags", bufs=4))
    batchp = ctx.enter_context(tc.tile_pool(name="batchp", bufs=2))
    psum_u = ctx.enter_context(tc.tile_pool(name="psum_u", bufs=1, space="PSUM"))
    psum_f = ctx.enter_context(tc.tile_pool(name="psum_f", bufs=2, space="PSUM"))
    psum_tr = ctx.enter_context(tc.tile_pool(name="psum_tr", bufs=2, space="PSUM"))

    id16 = singles.tile([128, 128], F16)
    nc.sync.dma_start(out=id16, in_=id16_d[:, :])
    id32 = singles.tile([128, 128], F32)
    nc.sync.dma_start(out=id32, in_=id32_d[:, :])
    wres = singles.tile([128, NL * HC], F32)
    nc.sync.dma_start(out=wres, in_=wres_d[:, :])
    bres = singles.tile([1, B * NL], F32)
    nc.sync.dma_start(out=bres, in_=bres_d[:, :])
    ones = singles.tile([128, 1], F32)
    nc.vector.memset(ones, 1.0)
    partials = singles.tile([128, B * NL], F32)
    logits_sb = singles.tile([1, B * NL], F32)

    chunks = [(0, 512), (512, 256)]

    def batch_sweep():
        for bb in range(B):
            _batch(tc, nc, hs_d, mask_d, hs_pool, work, small, diags, batchp,
                   psum_u, psum_f, psum_tr, id16, id32, wres, partials, chunks,
                   bb)

    if reps == 1:
        batch_sweep()
    else:
        with tc.For_i(0, reps, 1):
            batch_sweep()

    # ---- reduce partials over partitions with a ones-matmul; add bias ----
    lg_ps = psum_tr.tile([1, B * NL], F32, tag="tr")
    nc.tensor.matmul(lg_ps, ones, partials, start=True, stop=True)
    nc.vector.tensor_add(logits_sb, lg_ps, bres)
    nc.sync.dma_start(out=out_d[:, :], in_=logits_sb)


def _batch(tc, nc, hs_d, mask_d, hs_pool, work, small, diags, batchp, psum_u,
           psum_f, psum_tr, id16, id32, wres, partials, chunks, bb):
    if True:
        maxacc = batchp.tile([128, H], F32, tag="maxacc")
        nc.gpsimd.memset(maxacc, NEG_INF)

        for hh in range(NUNIT):
            # ---- load 12 layer tiles for 256 tokens, cast f32->fp16 ----
            hs16 = []
            for n in range(L):
                t = hs_pool.tile([128, X, H], F16, tag=f"hs{n}")
                src = hs_d[n, bb, hh * TOK:(hh + 1) * TOK, :].rearrange(
                    "(x p) h -> p x h", p=128)
                if CAST_DMA:
                    nc.gpsimd.dma_start(out=t, in_=src)
                else:
                    t32 = hs_pool.tile([128, X, H], F32, tag=f"hs32_{n}",
                                       bufs=1)
                    nc.sync.dma_start(out=t32, in_=src)
                    nc.scalar.copy(t, t32)
                hs16.append(t)
            msk = small.tile([128, X], F32, tag="msk")
            nc.sync.dma_start(
                out=msk,
                in_=mask_d[bb, hh * TOK:(hh + 1) * TOK].rearrange("(x p) -> p x", p=128))

            for x in range(X):
                # ---- u = sum_n hs_n  (PE identity accumulation) ----
                u_ps = psum_u.tile([128, H], F32, tag="u")
                for c0, cw in chunks:
                    for n in range(L):
                        nc.tensor.matmul(
                            u_ps[:, c0:c0 + cw], id16, hs16[n][:, x, c0:c0 + cw],
                            start=(n == 0), stop=(n == L - 1))
                u16 = work.tile([128, H], F16, tag="u16")
                nc.scalar.copy(u16, u_ps)

                # ---- scores_n = sum_h (hs_n/12)*u  (fused STT + accum) ----
                scores = small.tile([128, L], F32, tag="scores")
                scratch = work.tile([128, H], F16, tag="scratch")
                for n in range(L):
                    nc.vector.scalar_tensor_tensor(
                        out=scratch, in0=hs16[n][:, x, :], scalar=1.0 / L,
                        in1=u16, op0=OP.mult, op1=OP.mult,
                        accum_out=scores[:, n:n + 1])

                # ---- softmax over the 12 layers ----
                mx = small.tile([128, 1], F32, tag="mx")
                nc.vector.tensor_reduce(out=mx, in_=scores, axis=AX.X, op=OP.max)
                negmx = small.tile([128, 1], F32, tag="negmx")
                nc.vector.tensor_scalar_mul(negmx, mx, -1.0)
                exps = small.tile([128, L], F32, tag="exps")
                denom = small.tile([128, 1], F32, tag="denom")
                nc.scalar.activation(out=exps, in_=scores, func=ACT.Exp,
                                     bias=negmx[:, 0:1], scale=1.0,
                                     accum_out=denom[:, 0:1])
                recip = small.tile([128, 1], F32, tag="recip")
                nc.vector.reciprocal(recip, denom)
                pscale = small.tile([128, L], F32, tag="pscale")
                nc.vector.tensor_scalar_mul(pscale, exps, recip[:, 0:1])

                # ---- final = sum_n p_n*hs_n  (PE diag accumulation) ----
                fin_ps = psum_f.tile([128, H], F32, tag="fin")
                for n in range(L):
                    dg = diags.tile([128, 128], F16, tag="diag")
                    nc.scalar.mul(dg, id16, pscale[:, n:n + 1])
                    for c0, cw in chunks:
                        nc.tensor.matmul(
                            fin_ps[:, c0:c0 + cw], dg, hs16[n][:, x, c0:c0 + cw],
                            start=(n == 0), stop=(n == L - 1))

                # ---- masked running max over tokens ----
                nc.vector.scalar_tensor_tensor(
                    out=maxacc, in0=fin_ps, scalar=msk[:, x:x + 1], in1=maxacc,
                    op0=OP.mult, op1=OP.max)

        # ---- pooled[b] = cross-partition max via PE transpose ----
        pooled = batchp.tile([128, HC], F32, tag="pooled")
        for c in range(HC):
            ptr = psum_tr.tile([128, 128], F32, tag="tr")
            nc.tensor.transpose(ptr, maxacc[:, c * 128:(c + 1) * 128], id32)
            nc.vector.tensor_reduce(out=pooled[:, c:c + 1], in_=ptr,
                                    axis=AX.X, op=OP.max)

        # ---- logits partials: sum_h pooled*W per label ----
        sc6 = small.tile([128, HC], F32, tag="sc6")
        for l in range(NL):
            nc.vector.scalar_tensor_tensor(
                out=sc6, in0=pooled, scalar=1.0,
                in1=wres[:, l * HC:(l + 1) * HC], op0=OP.mult, op1=OP.mult,
                accum_out=partials[:, bb * NL + l:bb * NL + l + 1])


_NC_CACHE = None


def _get_nc():
    global _NC_CACHE
    if _NC_CACHE is None:
        _NC_CACHE = _build_nc()
    return _NC_CACHE


def kernel(hidden_states, mask, W, b):
    hidden_states = np.asarray(hidden_states, dtype=np.float32)
    mask = np.asarray(mask, dtype=np.float32)
    W = np.asarray(W, dtype=np.float32)
    b = np.asarray(b, dtype=np.float32)

    nc = _get_nc()

    # wres[p, l*HC+c] = W[l, c*128+p]
    wres = np.ascontiguousarray(
        W.reshape(NL, HC, 128).transpose(2, 0, 1).reshape(128, NL * HC))
    bres = np.ascontiguousarray(np.tile(b, B)[None, :])
    id16 = np.eye(128, dtype=np.float16)
    id32 = np.eye(128, dtype=np.float32)

    in_maps = []
    for ci in range(NCORES):
        in_maps.append({
            "hs": np.ascontiguousarray(hidden_states[1:, ci * B:(ci + 1) * B]),
            "mask": np.ascontiguousarray(mask[ci * B:(ci + 1) * B]),
            "wres": wres,
            "bres": bres,
            "id16": id16,
            "id32": id32,
        })

    trace = bool(int(os.environ.get("BASS_KERNEL_TRACE", "0")))
    res = run_bass_kernel_spmd(nc, in_maps, list(range(NCORES)), trace=trace)
    if trace and res.exec_time_ns is not None:
        print(f"HW exec time: {res.exec_time_ns} ns")
        kernel.last_exec_time_ns = res.exec_time_ns
    out = np.concatenate(
        [res.results[i]["out"].reshape(B, NL) for i in range(NCORES)], axis=0)
    return out


if __name__ == "__main__":
    rng = np.random.default_rng(0)
    hs = rng.standard_normal((13, BFULL, TSEQ, H), dtype=np.float32)
    mask = np.ones((BFULL, TSEQ), dtype=np.float32)
    W = rng.standard_normal((NL, H), dtype=np.float32) * 0.02
    b = np.zeros((NL,), dtype=np.float32)
    out = kernel(hidden_states=hs, mask=mask, W=W, b=b)
    print(out)
